# revision 1
# baseline (speedup 1.0000x reference)
"""Causal single-head attention (B=4, S=4096, D=768) on 8 TRN2 NeuronCores.

Sharding: core = (batch b = core//2, half h = core%2). Per batch, the 32
query blocks of 128 rows are split between the two cores in a
causally-balanced interleave: slot s (0..15) of core (b, h) handles query
rows [256*s + 128*h, 256*s + 128*h + 128).  Slots are grouped 4-at-a-time
(group t = slots 4t..4t+3, 512 query columns) and each group processes the
key window [0, 1024*(t+1)) -- identical program shape on every core; the
h-dependent causal boundary is handled by two data-driven [128,128]
multiplicative mask tiles (inputs), so a single NEFF runs SPMD on all 8
cores.

Layout trick: scores are computed transposed, St[k, q] (k on partitions),
so after exp the P tile is directly the lhsT of the P@V matmul -- no
on-chip transposes anywhere.  No max-subtraction is needed: scaled scores
are ~N(0,1) (max |z| ~ 7 over the whole problem), so exp never overflows
fp32, and softmax is shift-invariant so the result matches the reference.
The softmax denominator comes for free from a ones-column appended to V.

Precision: everything on-chip (streamed x/W and the Q/K/V/P residents)
is float16 -- same PE throughput (1 cycle/row) and SBUF bytes as bf16
with 8x finer mantissa, and every tensor here is bounded orders of
magnitude inside fp16 range (max |x| ~ 5.5, max |q| ~ 5.5, P <= e^7).
PSUM accumulation and the final normalization are fp32.  End-to-end max
error vs the fp32 reference is ~5.6e-4 of absmax.
"""

import math

import numpy as np

B, S, D = 4, 4096, 768
P = 128
DT = D // P            # 6 d-tiles
NK = S // P            # 32 key tiles
NG = 4                 # query groups per core
QG = 512               # query columns per group
NSLOT = 16             # 128-row query blocks per core
QW = NSLOT * P         # 2048 query rows per core
SCALE = 1.0 / math.sqrt(D)

F16 = np.float16

_CACHE = {}


def _build():
    import concourse.tile as tile
    from concourse import bacc, mybir

    f32 = mybir.dt.float32
    f32r = mybir.dt.float32r
    f16 = mybir.dt.float16
    Exp = mybir.ActivationFunctionType.Exp

    nc = bacc.Bacc(
        "TRN2",
        target_bir_lowering=False,
        debug=False,
        enable_asserts=False,
        num_devices=8,
    )

    xt = nc.dram_tensor("xt", [D, S], f16, kind="ExternalInput").ap()
    xq = nc.dram_tensor("xq", [D, QW], f16, kind="ExternalInput").ap()
    wq = nc.dram_tensor("wq", [D, D], f16, kind="ExternalInput").ap()
    wk = nc.dram_tensor("wk", [D, D], f16, kind="ExternalInput").ap()
    wv = nc.dram_tensor("wv", [D, D], f16, kind="ExternalInput").ap()
    xn = nc.dram_tensor("xn", [S, D], f16, kind="ExternalInput").ap()
    masks = nc.dram_tensor("masks", [2, P, P], f16, kind="ExternalInput").ap()
    out = nc.dram_tensor("out", [QW, D], f16, kind="ExternalOutput").ap()

    with tile.TileContext(nc, pool_alloc_mode="queue") as tc:
        with (
            tc.tile_pool(name="resid", bufs=1) as resid,
            tc.tile_pool(name="psS", bufs=3, space="PSUM") as psS,
            tc.tile_pool(name="utp", bufs=4, space="PSUM") as utp,
        ):
            kt = resid.tile([P, DT, S], f16)        # K^T  [d, keys]
            qt = resid.tile([P, DT, QW], f16)       # Q^T  [d, queries]
            xnat = resid.tile([P, NK, D], f16)       # x natural [keys, d]
            wv_r = resid.tile([P, DT, D], mybir.dt.float32r)  # Wv for final GEMM
            ones_sb = resid.tile([P, 1], f16)
            mask_sb = resid.tile([P, 2, P], f16)

            for r in range(2):
                nc.sync.dma_start(mask_sb[:, r, :], masks[r, :, :])
            for kk in range(NK):
                nc.sync.dma_start(xnat[:, kk, :], xn[kk * P : (kk + 1) * P, :])
            nc.vector.memset(ones_sb[:], 1.0)

            # ---------------- Phase 1: projections ----------------
            # Q^T[do, q] = sum_di Wq[di, do]^T x^T[di, q]
            with tc.tile_pool(name="wqp", bufs=1) as wqp, tc.tile_pool(
                name="xinq", bufs=3
            ) as xinq:
                wq_sb = wqp.tile([P, DT, D], f16)
                for di in range(DT):
                    nc.sync.dma_start(
                        wq_sb[:, di, :], wq[di * P : (di + 1) * P, :]
                    )
                for qc in range(QW // 512):
                    xch = xinq.tile([P, DT, 512], f16, tag="xin")
                    for di in range(DT):
                        nc.sync.dma_start(
                            xch[:, di, :],
                            xq[di * P : (di + 1) * P, qc * 512 : (qc + 1) * 512],
                        )
                    for do in range(DT):
                        ps = psS.tile([P, 512], f32)
                        for di in range(DT):
                            nc.tensor.matmul(
                                ps[:],
                                wq_sb[:, di, do * P : (do + 1) * P],
                                xch[:, di, :],
                                start=(di == 0),
                                stop=(di == DT - 1),
                            )
                        nc.vector.tensor_copy(
                            qt[:, do, qc * 512 : (qc + 1) * 512], ps[:]
                        )

            # K^T and V per 512-wide key chunk (x^T streamed once)
            with tc.tile_pool(name="wkv", bufs=1) as wkv, tc.tile_pool(
                name="xink", bufs=3
            ) as xink:
                wk_sb = wkv.tile([P, DT, D], f16, tag="wk")
                wv_sb = wkv.tile([P, DT, D], f16, tag="wv")
                for di in range(DT):
                    nc.sync.dma_start(
                        wv_sb[:, di, :], wv[di * P : (di + 1) * P, :]
                    )
                    nc.vector.tensor_copy(wv_r[:, di, :], wv_sb[:, di, :])
                for di in range(DT):
                    nc.sync.dma_start(
                        wk_sb[:, di, :], wk[di * P : (di + 1) * P, :]
                    )
                for kc in range(S // 512):
                    xch = xink.tile([P, DT, 512], f16, tag="xin")
                    for di in range(DT):
                        nc.sync.dma_start(
                            xch[:, di, :],
                            xt[di * P : (di + 1) * P, kc * 512 : (kc + 1) * 512],
                        )
                    for do in range(DT):
                        ps = psS.tile([P, 512], f32)
                        for di in range(DT):
                            nc.tensor.matmul(
                                ps[:],
                                wk_sb[:, di, do * P : (do + 1) * P],
                                xch[:, di, :],
                                start=(di == 0),
                                stop=(di == DT - 1),
                            )
                        nc.vector.tensor_copy(
                            kt[:, do, kc * 512 : (kc + 1) * 512], ps[:]
                        )
            # ------------- Phase 2: attention, reassociated values -------------
            # ctx = (P @ x) @ Wv: Ut = x^T P accumulated transpose-free in two
            # 3-bank di-sweeps over the resident P tiles; denominator l via an
            # ones-row matmul; final GEMM contracts over d for only the core's
            # 2048 queries.
            with (
                tc.tile_pool(name="ptp", bufs=34) as ptp,
                tc.tile_pool(name="utsb", bufs=8) as utsb,
                tc.tile_pool(name="outp", bufs=2) as outp,
                tc.tile_pool(name="small", bufs=4) as small,
            ):
                f32r = mybir.dt.float32r
                for t in range(NG):
                    win = 8 * t + 8
                    pts = []
                    c0s = []
                    ut_a = utp.tile([P, QG], f32, tag="ut")
                    ut_b = utp.tile([P, QG], f32, tag="ut")
                    ut_c = utp.tile([P, QG], f32, tag="ut")
                    ut_ps = [ut_a, ut_b, ut_c]
                    for k in range(win):
                        j0 = (k - 8 * t) // 2 if k - 8 * t >= 2 else 0
                        c0 = j0 * P
                        c0s.append(c0)
                        ps = psS.tile([P, QG], f32)
                        for di in range(DT):
                            nc.tensor.matmul(
                                ps[:, c0:QG],
                                kt[:, di, k * P : (k + 1) * P],
                                qt[:, di, t * QG + c0 : (t + 1) * QG],
                                start=(di == 0),
                                stop=(di == DT - 1),
                            )
                        pt = ptp.tile([P, QG], f16, tag="pt")
                        nc.scalar.activation(
                            pt[:, c0:QG], ps[:, c0:QG], Exp, scale=SCALE
                        )
                        if k >= 8 * t:
                            j = (k - 8 * t) // 2
                            rel = (k - 8 * t) % 2
                            nc.vector.tensor_mul(
                                pt[:, j * P : (j + 1) * P],
                                pt[:, j * P : (j + 1) * P],
                                mask_sb[:, rel, :],
                            )
                        pts.append(pt)
                        for di in range(3):
                            nc.tensor.matmul(
                                ut_ps[di][:, c0:QG],
                                xnat[:, k, di * P : (di + 1) * P],
                                pt[:, c0:QG],
                                start=(k == 0),
                                stop=(k == win - 1),
                            )
                    ut_sb = []
                    for di in range(3):
                        u = utsb.tile([P, QG], f32r, tag="ut_sb")
                        nc.vector.tensor_copy(u[:], ut_ps[di][:])
                        ut_sb.append(u)
                    ut_d = utp.tile([P, QG], f32, tag="ut")
                    ut_e = utp.tile([P, QG], f32, tag="ut")
                    ut_f = utp.tile([P, QG], f32, tag="ut")
                    ut_ps2 = [ut_d, ut_e, ut_f]
                    for k in range(win):
                        for di in range(3):
                            nc.tensor.matmul(
                                ut_ps2[di][:, c0s[k]:QG],
                                xnat[:, k, (di + 3) * P : (di + 4) * P],
                                pts[k][:, c0s[k]:QG],
                                start=(k == 0),
                                stop=(k == win - 1),
                            )
                    for di in range(3):
                        u = utsb.tile([P, QG], f32r, tag="ut_sb")
                        nc.vector.tensor_copy(u[:], ut_ps2[di][:])
                        ut_sb.append(u)
                    for j in range(4):
                        pso = utp.tile([P, 512], f32, tag="ut")
                        pso2f = utp.tile([P, 512], f32, tag="ut")
                        pso2 = pso2f[:, 0:256]
                        for di in range(DT):
                            nc.tensor.matmul(
                                pso[:],
                                ut_sb[di % 3 + (di // 3) * 3][:, j * P : (j + 1) * P],
                                wv_r[:, di, 0:512],
                                start=(di == 0),
                                stop=(di == DT - 1),
                            )
                        for di in range(DT):
                            nc.tensor.matmul(
                                pso2[:],
                                ut_sb[di][:, j * P : (j + 1) * P],
                                wv_r[:, di, 512:768],
                                start=(di == 0),
                                stop=(di == DT - 1),
                            )
                        nkj = 8 * t + 2 * j + 2
                        pslf = utp.tile([P, 512], f32, tag="ut")
                        psl = pslf[:, 0:1]
                        for k in range(nkj):
                            nc.tensor.matmul(
                                psl[:],
                                pts[k][:, j * P : (j + 1) * P],
                                ones_sb[:, 0:1],
                                start=(k == 0),
                                stop=(k == nkj - 1),
                            )
                        linv = small.tile([P, 1], f32, tag="linv")
                        nc.vector.reciprocal(linv[:], psl[:])
                        osb = outp.tile([P, D], f16, tag="osb")
                        nc.vector.tensor_scalar_mul(osb[:, 0:512], pso[:], linv[:])
                        nc.vector.tensor_scalar_mul(
                            osb[:, 512:768], pso2[:], linv[:]
                        )
                        s = 4 * t + j
                        nc.sync.dma_start(out[s * P : (s + 1) * P, :], osb[:])

    nc.compile()
    return nc


def _get_nc():
    if "nc" not in _CACHE:
        _CACHE["nc"] = _build()
    return _CACHE["nc"]


def _make_in_maps(x, Wq, Wk, Wv):
    x = np.asarray(x, dtype=np.float32)
    wq = np.ascontiguousarray(np.asarray(Wq, dtype=np.float32)).astype(F16)
    wk = np.ascontiguousarray(np.asarray(Wk, dtype=np.float32)).astype(F16)
    wv = np.ascontiguousarray(np.asarray(Wv, dtype=np.float32)).astype(F16)

    tri = (np.arange(P)[:, None] <= np.arange(P)[None, :]).astype(np.float32)
    ones = np.ones((P, P), dtype=np.float32)
    zeros = np.zeros((P, P), dtype=np.float32)
    mask_h = [
        np.stack([tri, zeros]).astype(F16),  # h=0: rel0 tri, rel1 zero
        np.stack([ones, tri]).astype(F16),   # h=1: rel0 ones, rel1 tri
    ]

    # x is uploaded as the zero-copy [8*QW, D] fp16 reshape (each core's own
    # query rows); xt/xq are derived on device by the prep function.
    xsh = np.ascontiguousarray(x.astype(F16).reshape(8 * QW, D))
    in_maps = []
    for core in range(8):
        h = core % 2
        in_maps.append(
            {
                "xsh": xsh,  # global array, shared entry
                "wq": wq,
                "wk": wk,
                "wv": wv,
                "masks": mask_h[h],
            }
        )
    return in_maps


def _get_exec():
    """Build (once) a cached jitted SPMD callable over 8 cores.

    Mirrors concourse.bass2jax.run_bass_via_pjrt's multi-core path, but keeps
    the jitted function so repeat calls skip retracing.
    """
    if "exec" in _CACHE:
        return _CACHE["exec"]

    import jax
    from jax.sharding import Mesh, PartitionSpec
    from jax.experimental.shard_map import shard_map
    import concourse.mybir as mybir
    from concourse.bass2jax import (
        _bass_exec_p,
        install_neuronx_cc_hook,
        partition_id_tensor,
    )

    install_neuronx_cc_hook()
    nc = _get_nc()
    partition_name = nc.partition_id_tensor.name if nc.partition_id_tensor else None

    in_names, out_names, out_avals, zero_shapes = [], [], [], []
    for alloc in nc.m.functions[0].allocations:
        if not isinstance(alloc, mybir.MemoryLocationSet):
            continue
        name = alloc.memorylocations[0].name
        if alloc.kind == "ExternalInput":
            if name == partition_name:
                continue
            in_names.append(name)
        elif alloc.kind == "ExternalOutput":
            out_names.append(name)
            shape = tuple(alloc.tensor_shape)
            dtype = mybir.dt.np(alloc.dtype)
            out_avals.append(jax.core.ShapedArray(shape, dtype))
            zero_shapes.append((shape, dtype))
    n_params = len(in_names)
    n_outs = len(out_avals)
    all_names = in_names + out_names
    if partition_name is not None:
        all_names = all_names + [partition_name]
    donate = tuple(range(n_params, n_params + n_outs))

    def _body(*args):
        operands = list(args)
        if partition_name is not None:
            operands.append(partition_id_tensor())
        outs = _bass_exec_p.bind(
            *operands,
            out_avals=tuple(out_avals),
            in_names=tuple(all_names),
            out_names=tuple(out_names),
            lowering_input_output_aliases=(),
            sim_require_finite=True,
            sim_require_nnan=True,
            nc=nc,
        )
        return tuple(outs)

    devices = jax.devices()[:8]
    mesh = Mesh(np.asarray(devices), ("core",))
    # Weights are identical on every core: replicate instead of sharding so
    # they are uploaded once per call instead of 8x.
    replicated = {"wq", "wk", "wv"}
    in_specs = tuple(
        PartitionSpec() if name in replicated else PartitionSpec("core")
        for name in in_names
    ) + (PartitionSpec("core"),) * n_outs
    sharded = jax.jit(
        shard_map(
            _body,
            mesh=mesh,
            in_specs=in_specs,
            out_specs=(PartitionSpec("core"),) * n_outs,
            check_rep=False,
        ),
        donate_argnums=donate,
        keep_unused=True,
    )

    # On-device input prep (saves shipping 75MB/call): each core uploads only
    # its own 2048-row slice of x; a pairwise all_gather reconstructs the
    # batch's full [4096, 768] sequence, which is transposed to x^T and the
    # core's query columns gathered -- all device-side.
    def _prep_inputs(x_shard):
        import jax.numpy as jnp
        from jax import lax

        h = lax.axis_index("core") % 2
        x_full = lax.all_gather(
            x_shard,
            "core",
            axis_index_groups=[[0, 1], [2, 3], [4, 5], [6, 7]],
            axis=0,
            tiled=True,
        )  # [S, D]
        xt = jnp.transpose(x_full)  # [D, S]
        xqrows = lax.dynamic_slice_in_dim(
            x_full.reshape(NSLOT, 2, P, D), h, 1, axis=1
        ).reshape(QW, D)
        xq = jnp.transpose(xqrows)  # [D, QW]
        return xt, xq, x_full

    prep = jax.jit(
        shard_map(
            _prep_inputs,
            mesh=mesh,
            in_specs=(PartitionSpec("core"),),
            out_specs=(PartitionSpec("core"),) * 3,
            check_rep=False,
        )
    )
    _CACHE["exec"] = (
        sharded, in_names, out_names, out_avals, zero_shapes, replicated, prep, mesh,
    )
    return _CACHE["exec"]


def _concat_inputs(in_maps, in_names, replicated=frozenset(("wq", "wk", "wv"))):
    return [
        np.asarray(in_maps[0][name])
        if name in replicated
        else np.concatenate([np.asarray(m[name]) for m in in_maps], axis=0)
        for name in in_names
    ]


def _make_zeros(zero_shapes):
    return [
        np.zeros((8 * shape[0], *shape[1:]), dtype) for shape, dtype in zero_shapes
    ]


def _run(in_maps):
    import jax
    from jax.sharding import NamedSharding, PartitionSpec

    (sharded, in_names, out_names, out_avals, zero_shapes, replicated,
     prep, mesh) = _get_exec()
    xt_dev, xq_dev, xn_dev = prep(in_maps[0]["xsh"])
    staged = {"xt": xt_dev, "xq": xq_dev, "xn": xn_dev}
    concat_in = [
        staged[name] if name in staged
        else _concat_inputs(in_maps, [name], replicated)[0]
        for name in in_names
    ]
    # The kernel writes every output element, so the donated output buffers
    # never need zeroing; reuse the previous call's device-resident outputs
    # instead of shipping fresh zero arrays each call.
    donated = _CACHE.pop("outbuf", None)
    if donated is None:
        donated = _make_zeros(zero_shapes)
    out_arrs = sharded(*concat_in, *donated)
    _CACHE["outbuf"] = list(out_arrs)
    i = out_names.index("out")
    full = np.asarray(out_arrs[i]).reshape(8, *out_avals[i].shape)
    return [full[c] for c in range(8)]


def kernel(x, Wq, Wk, Wv):
    in_maps = _make_in_maps(x, Wq, Wk, Wv)
    outs = _run(in_maps)
    out = np.empty((B, S, D), dtype=np.float32)
    for core in range(8):
        b, h = core // 2, core % 2
        out[b].reshape(NSLOT, 2, P, D)[:, h] = outs[core].reshape(NSLOT, P, D)
    return out



# revision 9
# speedup vs baseline: 1.9184x; 1.9184x over previous
"""Causal single-head attention (B=4, S=4096, D=768) on 8 TRN2 NeuronCores.

Sharding (as the f16 baseline): core = (batch b = core//2, half h = core%2).
Per batch, the 32 query blocks of 128 rows are split between the two cores in
a causally-balanced interleave: slot s (0..15) of core (b, h) handles query
rows [256*s + 128*h, 256*s + 128*h + 128).  Slots are grouped 4-at-a-time
(group t, 512 query columns) over the key window [0, 1024*(t+1)); the
h-dependent causal boundary is handled by data-driven multiplicative mask
tiles, so a single NEFF runs SPMD on all 8 cores.

Arithmetic: fp8e4 (e4m3) matmuls in DoubleRow perf mode (two 128-deep
contraction slices per instruction at 0.5 cycles/output-column -- 4x the
fp16 rate under the TRN2 cost model).  Precision is held inside the 2e-2
budget by hi/lo error compensation:

  * Wq@Wk^T is folded on the host into one matrix M, so scores need no
    K projection at all: z = (x@M) @ x^T.
  * y = x@M is computed 3-term exact-grade: xh@Mh + xl@Mh + xh@Ml, where
    *_h is the fp8 quantization and *_l the fp8-quantized residual.
  * scores are 2-term: (yh + yl) . xth  (y exact-grade, key-side x straight
    fp8).  P = exp(z*scale - 2) is written by the activation directly to
    fp8 (the -2 shift keeps P in [0, e^5] -- far from e4m3's 240 max, and
    softmax is shift-invariant; the denominator uses the same quantized P).
  * Ut = x^T P is 2-term: (xnh + xnl) . P8, DoubleRow-paired over key tiles.
  * the final GEMM (U^T @ Wv) and the normalization stay f32r/f32 -- Wv
    quantization error would hit the output unsuppressed.
  * the softmax-reweighting errors of the two straight-fp8 operands (key-side
    x, P) are suppressed by sqrt(sum w^2) everywhere except the first rows
    with few keys; the (t=0, j=0) block (global rows 0..255 across the two
    h-cores) is therefore recomputed 4-term exact-grade and substituted at
    output time.  Measured end-to-end: ~1.1e-2 max rel err (budget 2e-2).

Denominators come from DoubleRow ones-matmuls over the resident P pairs.
"""

import math

import numpy as np
import ml_dtypes

B, S, D = 4, 4096, 768
P = 128
DT = D // P            # 6 d-tiles
NK = S // P            # 32 key tiles
NG = 4                 # query groups per core
QG = 512               # query columns per group
NSLOT = 16             # 128-row query blocks per core
QW = NSLOT * P         # 2048 query rows per core
SCALE = 1.0 / math.sqrt(D)
PBIAS = -2.0           # exp shift: P in (0, e^5], safe in e4m3

E4 = ml_dtypes.float8_e4m3
F16 = np.float16

_CACHE = {}


def _build():
    import concourse.tile as tile
    from concourse import bacc, mybir

    f32 = mybir.dt.float32
    f32r = mybir.dt.float32r
    f16 = mybir.dt.float16
    f8 = mybir.dt.float8e4
    DR = mybir.MatmulPerfMode.DoubleRow
    Exp = mybir.ActivationFunctionType.Exp

    nc = bacc.Bacc(
        "TRN2",
        target_bir_lowering=False,
        debug=False,
        enable_asserts=False,
        num_devices=8,
    )

    xth = nc.dram_tensor("xth", [DT, P, S], f8, kind="ExternalInput").ap()
    xtl = nc.dram_tensor("xtl", [DT, P, S], f8, kind="ExternalInput").ap()
    xnh = nc.dram_tensor("xnh", [NK, P, D], f8, kind="ExternalInput").ap()
    xnl = nc.dram_tensor("xnl", [NK, P, D], f8, kind="ExternalInput").ap()
    xqh = nc.dram_tensor("xqh", [DT, P, QW], f8, kind="ExternalInput").ap()
    xql = nc.dram_tensor("xql", [DT, P, QW], f8, kind="ExternalInput").ap()
    mh = nc.dram_tensor("mh", [DT, P, D], f8, kind="ExternalInput").ap()
    ml = nc.dram_tensor("ml", [DT, P, D], f8, kind="ExternalInput").ap()
    wv = nc.dram_tensor("wv", [DT, P, D], f32r, kind="ExternalInput").ap()
    masks = nc.dram_tensor("masks", [2, P, P], f8, kind="ExternalInput").ap()
    out = nc.dram_tensor("out", [QW, D], f16, kind="ExternalOutput").ap()

    with tile.TileContext(nc, pool_alloc_mode="queue") as tc:
        with (
            tc.tile_pool(name="resid", bufs=1) as resid,
            tc.tile_pool(name="psS", bufs=3, space="PSUM") as psS,
            tc.tile_pool(name="utp", bufs=4, space="PSUM") as utp,
            tc.tile_pool(name="pspt", bufs=1, space="PSUM") as pspt,
        ):
            xt_h = resid.tile([P, DT, S], f8)
            xt_l = resid.tile([P, DT, S], f8)
            xn_h = resid.tile([P, NK, D], f8)
            xn_l = resid.tile([P, NK, D], f8)
            y_h = resid.tile([P, DT, QW], f8)
            y_l = resid.tile([P, DT, QW], f8)
            wv_r = resid.tile([P, DT, D], f32r)
            ones8 = resid.tile([P, 2, 1], f8)
            mask_sb = resid.tile([P, 2, P], f8)
            bias_sb = resid.tile([P, 1], f32)

            nc.vector.memset(ones8[:], 1.0)
            nc.vector.memset(bias_sb[:], PBIAS)
            nc.sync.dma_start(mask_sb[:], masks.rearrange("r p c -> p r c"))
            nc.sync.dma_start(wv_r[:], wv.rearrange("d p c -> p d c"))

            # ---------------- Phase 1: y = x @ M (3-term fp8) --------------
            with tc.tile_pool(name="mp", bufs=1) as mp, tc.tile_pool(
                name="xinq", bufs=2
            ) as xinq:
                m_h = mp.tile([P, DT, D], f8, tag="mh")
                m_l = mp.tile([P, DT, D], f8, tag="ml")
                xq_sh = mp.tile([P, DT, QW], f8, tag="xqsh")
                xq_sl = mp.tile([P, DT, QW], f8, tag="xqsl")
                nc.sync.dma_start(m_h[:], mh.rearrange("d p c -> p d c"))
                nc.sync.dma_start(m_l[:], ml.rearrange("d p c -> p d c"))
                nc.sync.dma_start(xq_sh[:], xqh.rearrange("d p q -> p d q"))
                nc.sync.dma_start(xq_sl[:], xql.rearrange("d p q -> p d q"))
                for qc in range(QW // QG):
                    xq_h = xq_sh[:, :, qc * QG:(qc + 1) * QG]
                    xq_l = xq_sl[:, :, qc * QG:(qc + 1) * QG]
                    for do in range(DT):
                        ps = psS.tile([P, QG], f32)
                        terms = [(m_h, xq_h), (m_h, xq_l), (m_l, xq_h)]
                        n = 0
                        for mt, xt_ in terms:
                            for dp in range(DT // 2):
                                nc.tensor.matmul(
                                    ps[:],
                                    mt[:, 2 * dp:2 * dp + 2, do * P:(do + 1) * P],
                                    xt_[:, 2 * dp:2 * dp + 2, :],
                                    start=(n == 0),
                                    stop=(n == 3 * DT // 2 - 1),
                                    perf_mode=DR,
                                )
                                n += 1
                        # split psum into fp8 hi/lo: hi on Scalar, rest on DVE
                        ydf = xinq.tile([P, QG], f32, tag="ydf")
                        yhs = y_h[:, do, qc * QG:(qc + 1) * QG]
                        nc.scalar.copy(yhs, ps[:])
                        nc.vector.tensor_sub(ydf[:], ps[:], yhs)
                        nc.vector.tensor_copy(
                            y_l[:, do, qc * QG:(qc + 1) * QG], ydf[:]
                        )

            # load the big residents (batched single DMAs)
            nc.sync.dma_start(xt_h[:], xth.rearrange("d p s -> p d s"))
            nc.sync.dma_start(xt_l[:], xtl.rearrange("d p s -> p d s"))
            nc.sync.dma_start(xn_h[:], xnh.rearrange("k p d -> p k d"))
            nc.sync.dma_start(xn_l[:], xnl.rearrange("k p d -> p k d"))

            # ------------- Phase 2: attention, reassociated values ----------
            with (
                tc.tile_pool(name="ptp", bufs=17) as ptp,
                tc.tile_pool(name="utsb", bufs=8) as utsb,
                tc.tile_pool(name="outp", bufs=2) as outp,
                tc.tile_pool(name="small", bufs=4) as small,
                tc.tile_pool(name="patch", bufs=1) as patch,
            ):
                # exact-grade patch tiles for the (t=0, j=0) block
                pp32 = patch.tile([P, 2, P], f32)
                pp_h = patch.tile([P, 2, P], f8)
                pp_l = patch.tile([P, 2, P], f8)
                ppdf = patch.tile([P, 2, P], f32)
                utp_a = pspt.tile([P, QG], f32)   # patch Ut psum (A then B)
                utpsb = patch.tile([P, 2, 3 * P], f32r)
                pslp = utp_a[:, 448:449]   # spare psum cells in the same bank

                for t in range(NG):
                    npair = 4 * t + 4
                    pts = []
                    c0s = []
                    ut_ps = [
                        utp.tile([P, QG], f32, tag="ut", name=f"utA{t}_{i}")
                        for i in range(3)
                    ]
                    for kp in range(npair):
                        rel = kp - 4 * t  # pair index within diagonal region
                        c0 = rel * P if rel >= 1 else 0
                        c0s.append(c0)
                        pt = ptp.tile([P, 2, QG], f8, tag="pt")
                        for ki in range(2):
                            k = 2 * kp + ki
                            ps = psS.tile([P, QG], f32)
                            n = 0
                            for ysrc in (y_h, y_l):
                                for dp in range(DT // 2):
                                    nc.tensor.matmul(
                                        ps[:, c0:QG],
                                        xt_h[:, 2 * dp:2 * dp + 2,
                                             k * P:(k + 1) * P],
                                        ysrc[:, 2 * dp:2 * dp + 2,
                                             t * QG + c0:(t + 1) * QG],
                                        start=(n == 0),
                                        stop=(n == DT - 1),
                                        perf_mode=DR,
                                    )
                                    n += 1
                            nc.scalar.activation(
                                pt[:, ki, c0:QG], ps[:, c0:QG], Exp,
                                scale=SCALE, bias=bias_sb[:],
                            )
                            if rel >= 0:
                                # diagonal block j == rel gets the causal mask
                                nc.vector.tensor_mul(
                                    pt[:, ki, rel * P:(rel + 1) * P],
                                    pt[:, ki, rel * P:(rel + 1) * P],
                                    mask_sb[:, ki, :],
                                )
                        pts.append(pt)
                        for di in range(3):
                            for xns in (xn_h, xn_l):
                                nc.tensor.matmul(
                                    ut_ps[di][:, c0:QG],
                                    xns[:, 2 * kp:2 * kp + 2,
                                        di * P:(di + 1) * P],
                                    pt[:, :, c0:QG],
                                    start=(kp == 0 and xns is xn_h),
                                    stop=(kp == npair - 1 and xns is xn_l),
                                    perf_mode=DR,
                                )
                        if t == 0 and kp == 0:
                            # ---- exact-grade patch for block (t=0, j=0) ----
                            for ki in range(2):
                                k = ki
                                psp = utp.tile([P, QG], f32, tag="ut")
                                n = 0
                                for xts in (xt_h, xt_l):
                                    for ysrc in (y_h, y_l):
                                        for dp in range(DT // 2):
                                            nc.tensor.matmul(
                                                psp[:, 0:P],
                                                xts[:, 2 * dp:2 * dp + 2,
                                                    k * P:(k + 1) * P],
                                                ysrc[:, 2 * dp:2 * dp + 2,
                                                     0:P],
                                                start=(n == 0),
                                                stop=(n == 2 * DT - 1),
                                                perf_mode=DR,
                                            )
                                            n += 1
                                nc.scalar.activation(
                                    pp32[:, ki, :], psp[:, 0:P], Exp,
                                    scale=SCALE, bias=bias_sb[:],
                                )
                                nc.vector.tensor_mul(
                                    pp32[:, ki, :], pp32[:, ki, :],
                                    mask_sb[:, ki, :],
                                )
                            nc.scalar.copy(pp_h[:], pp32[:])
                            nc.vector.tensor_sub(ppdf[:], pp32[:], pp_h[:])
                            nc.vector.tensor_copy(pp_l[:], ppdf[:])
                            for i, pp in enumerate((pp_h, pp_l)):
                                nc.tensor.matmul(
                                    pslp, pp[:, :, :], ones8[:],
                                    start=(i == 0), stop=(i == 1),
                                    perf_mode=DR,
                                )
                            for half in range(2):
                                n = 0
                                for di in range(3):
                                    d = 3 * half + di
                                    for xns in (xn_h, xn_l):
                                        for pp in (pp_h, pp_l):
                                            nc.tensor.matmul(
                                                utp_a[:, di * P:(di + 1) * P],
                                                xns[:, 0:2, d * P:(d + 1) * P],
                                                pp[:, :, :],
                                                start=(n % 4 == 0),
                                                stop=(n % 4 == 3),
                                                perf_mode=DR,
                                            )
                                            n += 1
                                nc.vector.tensor_copy(
                                    utpsb[:, half, :], utp_a[:, 0:3 * P]
                                )
                    ut_sb = []
                    for di in range(3):
                        u = utsb.tile([P, QG], f32r, tag="ut_sb")
                        nc.vector.tensor_copy(u[:], ut_ps[di][:])
                        ut_sb.append(u)
                    ut_ps2 = [
                        utp.tile([P, QG], f32, tag="ut", name=f"utB{t}_{i}")
                        for i in range(3)
                    ]
                    for kp in range(npair):
                        for di in range(3):
                            for xns in (xn_h, xn_l):
                                nc.tensor.matmul(
                                    ut_ps2[di][:, c0s[kp]:QG],
                                    xns[:, 2 * kp:2 * kp + 2,
                                        (di + 3) * P:(di + 4) * P],
                                    pts[kp][:, :, c0s[kp]:QG],
                                    start=(kp == 0 and xns is xn_h),
                                    stop=(kp == npair - 1 and xns is xn_l),
                                    perf_mode=DR,
                                )
                    for di in range(3):
                        u = utsb.tile([P, QG], f32r, tag="ut_sb")
                        nc.vector.tensor_copy(u[:], ut_ps2[di][:])
                        ut_sb.append(u)
                    for j in range(4):
                        patched = (t == 0 and j == 0)
                        pso = utp.tile([P, QG], f32, tag="ut")
                        pso2f = utp.tile([P, QG], f32, tag="ut")
                        pso2 = pso2f[:, 0:256]
                        if patched:
                            def lhs(di):
                                return utpsb[:, di // 3, (di % 3) * P:
                                             (di % 3 + 1) * P]
                        else:
                            def lhs(di):
                                return ut_sb[di][:, j * P:(j + 1) * P]
                        for di in range(DT):
                            nc.tensor.matmul(
                                pso[:], lhs(di), wv_r[:, di, 0:QG],
                                start=(di == 0), stop=(di == DT - 1),
                            )
                        for di in range(DT):
                            nc.tensor.matmul(
                                pso2[:], lhs(di), wv_r[:, di, QG:D],
                                start=(di == 0), stop=(di == DT - 1),
                            )
                        linv = small.tile([P, 1], f32, tag="linv")
                        if patched:
                            nc.vector.reciprocal(linv[:], pslp)
                        else:
                            npj = 4 * t + j + 1   # pairs in this block's window
                            pslf = utp.tile([P, QG], f32, tag="ut")
                            psl = pslf[:, 0:1]
                            for kp in range(npj):
                                nc.tensor.matmul(
                                    psl[:],
                                    pts[kp][:, :, j * P:(j + 1) * P],
                                    ones8[:],
                                    start=(kp == 0), stop=(kp == npj - 1),
                                    perf_mode=DR,
                                )
                            nc.vector.reciprocal(linv[:], psl[:])
                        osb = outp.tile([P, D], f16, tag="osb")
                        nc.vector.tensor_scalar_mul(osb[:, 0:QG], pso[:], linv[:])
                        nc.vector.tensor_scalar_mul(
                            osb[:, QG:D], pso2[:], linv[:]
                        )
                        s = 4 * t + j
                        nc.sync.dma_start(out[s * P:(s + 1) * P, :], osb[:])

    nc.compile()
    return nc


def _get_nc():
    if "nc" not in _CACHE:
        _CACHE["nc"] = _build()
    return _CACHE["nc"]


def _hl(a):
    h = a.astype(E4)
    l = (a - h.astype(np.float32)).astype(E4)
    return h, l


def _make_in_maps(x, Wq, Wk, Wv):
    x = np.asarray(x, dtype=np.float32)
    Wq = np.asarray(Wq, dtype=np.float32)
    Wk = np.asarray(Wk, dtype=np.float32)
    Wv = np.asarray(Wv, dtype=np.float32)

    M = Wq @ Wk.T
    m_h, m_l = _hl(M)
    wv32 = np.ascontiguousarray(Wv)

    tri = (np.arange(P)[:, None] <= np.arange(P)[None, :]).astype(np.float32)
    ones = np.ones((P, P), dtype=np.float32)
    zeros = np.zeros((P, P), dtype=np.float32)
    mask_h = [
        np.stack([tri, zeros]).astype(E4),  # h=0: rel0 tri, rel1 zero
        np.stack([ones, tri]).astype(E4),   # h=1: rel0 ones, rel1 tri
    ]

    xh, xl = _hl(x)                          # [B, S, D] fp8 pair
    xhf = xh.astype(np.float32)
    xlf = xl.astype(np.float32)

    in_maps = []
    for core in range(8):
        b, h = core // 2, core % 2
        xth = np.ascontiguousarray(xhf[b].T).astype(E4).reshape(DT, P, S)
        xtl = np.ascontiguousarray(xlf[b].T).astype(E4).reshape(DT, P, S)
        # query columns of this core: rows 256*s + 128*h .. +128
        qrows = xhf[b].reshape(NSLOT, 2, P, D)[:, h].reshape(QW, D)
        qrows_l = xlf[b].reshape(NSLOT, 2, P, D)[:, h].reshape(QW, D)
        in_maps.append(
            {
                "xth": xth,
                "xtl": xtl,
                "xnh": xh[b].reshape(NK, P, D),
                "xnl": xl[b].reshape(NK, P, D),
                "xqh": np.ascontiguousarray(qrows.T).astype(E4).reshape(DT, P, QW),
                "xql": np.ascontiguousarray(qrows_l.T).astype(E4).reshape(DT, P, QW),
                "mh": m_h.reshape(DT, P, D),
                "ml": m_l.reshape(DT, P, D),
                "wv": wv32.reshape(DT, P, D),
                "masks": mask_h[h],
            }
        )
    return in_maps


def _get_exec():
    """Build (once) a cached jitted SPMD callable over 8 cores."""
    if "exec" in _CACHE:
        return _CACHE["exec"]

    import jax
    from jax.sharding import Mesh, PartitionSpec
    from jax.experimental.shard_map import shard_map
    import concourse.mybir as mybir
    from concourse.bass2jax import (
        _bass_exec_p,
        install_neuronx_cc_hook,
        partition_id_tensor,
    )

    install_neuronx_cc_hook()
    nc = _get_nc()
    partition_name = nc.partition_id_tensor.name if nc.partition_id_tensor else None

    in_names, out_names, out_avals, zero_shapes = [], [], [], []
    for alloc in nc.m.functions[0].allocations:
        if not isinstance(alloc, mybir.MemoryLocationSet):
            continue
        name = alloc.memorylocations[0].name
        if alloc.kind == "ExternalInput":
            if name == partition_name:
                continue
            in_names.append(name)
        elif alloc.kind == "ExternalOutput":
            out_names.append(name)
            shape = tuple(alloc.tensor_shape)
            dtype = mybir.dt.np(alloc.dtype)
            out_avals.append(jax.core.ShapedArray(shape, dtype))
            zero_shapes.append((shape, dtype))
    n_params = len(in_names)
    n_outs = len(out_avals)
    all_names = in_names + out_names
    if partition_name is not None:
        all_names = all_names + [partition_name]
    donate = tuple(range(n_params, n_params + n_outs))

    def _body(*args):
        operands = list(args)
        if partition_name is not None:
            operands.append(partition_id_tensor())
        outs = _bass_exec_p.bind(
            *operands,
            out_avals=tuple(out_avals),
            in_names=tuple(all_names),
            out_names=tuple(out_names),
            lowering_input_output_aliases=(),
            sim_require_finite=True,
            sim_require_nnan=True,
            nc=nc,
        )
        return tuple(outs)

    devices = jax.devices()[:8]
    mesh = Mesh(np.asarray(devices), ("core",))
    replicated = {"mh", "ml", "wv"}
    in_specs = tuple(
        PartitionSpec() if name in replicated else PartitionSpec("core")
        for name in in_names
    ) + (PartitionSpec("core"),) * n_outs
    sharded = jax.jit(
        shard_map(
            _body,
            mesh=mesh,
            in_specs=in_specs,
            out_specs=(PartitionSpec("core"),) * n_outs,
            check_rep=False,
        ),
        donate_argnums=donate,
        keep_unused=True,
    )
    _CACHE["exec"] = (
        sharded, in_names, out_names, out_avals, zero_shapes, replicated,
    )
    return _CACHE["exec"]


def _concat_inputs(in_maps, in_names, replicated=frozenset(("mh", "ml", "wv"))):
    return [
        np.asarray(in_maps[0][name])
        if name in replicated
        else np.concatenate([np.asarray(m[name]) for m in in_maps], axis=0)
        for name in in_names
    ]


def _make_zeros(zero_shapes):
    return [
        np.zeros((8 * shape[0], *shape[1:]), dtype) for shape, dtype in zero_shapes
    ]


def _run(in_maps):
    import jax

    (sharded, in_names, out_names, out_avals, zero_shapes, replicated) = _get_exec()
    concat_in = _concat_inputs(in_maps, in_names, replicated)
    donated = _CACHE.pop("outbuf", None)
    if donated is None:
        donated = _make_zeros(zero_shapes)
    out_arrs = sharded(*concat_in, *donated)
    _CACHE["outbuf"] = list(out_arrs)
    i = out_names.index("out")
    full = np.asarray(out_arrs[i]).reshape(8, *out_avals[i].shape)
    return [full[c] for c in range(8)]


def kernel(x, Wq, Wk, Wv):
    in_maps = _make_in_maps(x, Wq, Wk, Wv)
    outs = _run(in_maps)
    out = np.empty((B, S, D), dtype=np.float32)
    for core in range(8):
        b, h = core // 2, core % 2
        out[b].reshape(NSLOT, 2, P, D)[:, h] = outs[core].reshape(NSLOT, P, D)
    return out
